# revision 90
# baseline (speedup 1.0000x reference)
"""Banded multi-head attention (window=256) on 8 Trainium2 NeuronCores.

Sharding: core c handles batch b = c // 4 and head group g = c % 4
(4 of 16 heads). QKV projection is column-sharded per head group, the
banded attention is embarrassingly parallel over (batch, head), and the
output projection is row-sharded (each core produces a partial [S, E]
bf16 output; the host sums the 4 partials per batch in f32 + bias).

All matmul operands are bf16 (fp32 PSUM accumulation): 1 cycle/row at
any moving size, half the DMA bytes, and 2-4x DVE modes for the
element-wise tail. Padded tokens are zeroed on the HOST (x columns), so
no on-device keep-masking is needed (valid because bqkv == 0; a general
keep-mask + bias-lane variant builds when bqkv != 0).

Phase 1 (7-bank psum scope; the 8th bank holds the hoisted v-proj pool
so phase 2's first chains skip the pool-release boundary): the qk^T
projection chains run TILE-MAJOR over the first SPLIT contraction tiles
- every arriving (wqv_i, x_i) DMA pair feeds ~1.5-1.7us of matmuls,
covering the ~1.3us DMA cadence so the PE never starves during the load
- then c-major so each chain's eviction (ACT/DVE alternating) hides
behind the next chain's matmuls; the single-bank c3 chain's first
sub-chain completes early in that order so its eviction drains behind
c2/c1's matmuls and the second sub-chain never head-blocks the PE wait
queue. ldweights "touchers" (zero-cost PE instructions reading one
element per DMA) absorb the DMA semaphores one at a time so no matmul
carries a hoisted wait-for-all prefix.

Phase 2 software pipeline (per step t, the sim's 4-deep wait queue +
32-deep exec queue reorders around short stalls):
  scores(t-1), v_proj(t), AV(t-2), transpose+o_proj+store(t-3)
so the scores->exp->mask->AV cross-engine chain has a full step of
slack instead of stalling the PE.
  - scores per key block kb: 4 heads into two [128, 2, 512] paired psum
    tiles, 2 ACT exp (scale=1/8) -> p_sb bf16 [128, 4, 384], 1 DVE
    band-mask multiply on the outer q-thirds only (the middle 128
    columns of a key block's 384-query window are always in-band).
  - AV per q block: 12 matmuls (4h x 3kb) N=66 accumulate vals + the
    softmax denominator (ones lane in v_sb).
  - DVE recip + per-head normalize -> vals bf16; vals -> vT transpose
    via the DMA XBAR (dma_start transpose=True, 2 ops/block) for blocks
    0..NB-4 - the pipeline slack hides the ~2.5us DMA latency and it
    frees the PE transposes + DVE copy; the latency-critical tail
    blocks keep 2 PE transposes (bf16) aimed into the o_proj psum
    tile's bank 0 via a bf16 bitcast (no extra bank) + one DVE copy.
    Then 4 o_proj matmuls (bank 1 first - bank 0 waits the vT read),
    ACT eviction -> ot bf16, DMA out.
  - tail: the last o_proj blocks alternate onto the then-free scores
    psum ring so the drain chains run two-wide; the very last block
    evicts + stores per bank so the final DMA starts early.

PSUM budget phase 2 (8 banks): v 1, sc [128,2,512] bufs=2 = 4, av 1,
o_proj (transposes folded in) [128,2,512] bufs=1 = 2.

Cost-model time: 80924 ns (baseline 118130); HW rel err ~4.5e-3.
"""

import os

import numpy as np

B = 2
S = 2048
IN_DIM = 1024
EMBED = 1024
HEADS = 16
WINDOW = 256
HD = 64
H_LOC = 4          # heads per core
N_CORES = 8
QK_CH = 2 * H_LOC * HD   # 512
V_CH = H_LOC * HD        # 256
NB = S // 128            # 16 token blocks
VW = 66                  # 64 ch + ones (denominator) lane + pad

_CACHE = {}
LAST = {"exec_time_ns": None, "results": None}


def _build_nc(has_bias):
    import concourse.mybir as mybir
    import concourse.tile as tile
    from concourse import bacc
    from concourse.masks import make_identity
    import concourse.bass as bass
    from contextlib import ExitStack

    F32 = mybir.dt.float32
    BF16 = mybir.dt.bfloat16
    EXP = mybir.ActivationFunctionType.Exp
    COPY = mybir.ActivationFunctionType.Copy

    IN_ROWS = IN_DIM + (2 if has_bias else 0)
    KT = 9 if has_bias else 8

    def _rh(i):
        return min(128, IN_ROWS - 128 * i)

    nc = bacc.Bacc()

    xT = nc.dram_tensor("xT", [IN_ROWS, S], BF16, kind="ExternalInput")
    wqvT = nc.dram_tensor("wqvT", [IN_ROWS, QK_CH + V_CH], BF16, kind="ExternalInput")
    woT = nc.dram_tensor("woT", [V_CH, EMBED], BF16, kind="ExternalInput")
    mask01 = nc.dram_tensor("mask01", [128, 384], BF16, kind="ExternalInput")
    if has_bias:
        keep = nc.dram_tensor("keep", [1, S], F32, kind="ExternalInput")
    out = nc.dram_tensor("out", [S, EMBED], BF16, kind="ExternalOutput")

    with tile.TileContext(nc) as tc, ExitStack() as es:
        main = es.enter_context(tc.tile_pool(name="main", bufs=1))
        xpool = es.enter_context(tc.tile_pool(name="xpool", bufs=1))
        wk = es.enter_context(tc.tile_pool(name="wk", bufs=4))
        wk2 = es.enter_context(tc.tile_pool(name="wk2", bufs=2))

        # ---- persistent SBUF tiles ----
        ident = main.tile([128, 128], BF16)
        make_identity(nc, ident)
        mk = main.tile([128, 384], BF16)
        wo_t = [main.tile([128, EMBED], BF16, name=f"wo{c}") for c in range(2)]
        xt = [xpool.tile([_rh(i), S], BF16, name=f"xt{i}") for i in range(KT)]
        wqv_t = [
            xpool.tile([_rh(i), QK_CH + V_CH], BF16, name=f"wqv{i}") for i in range(KT)
        ]
        qk = [main.tile([128, S], BF16, name=f"qk{c}") for c in range(4)]
        v_sb = main.tile([128, NB, H_LOC, VW], BF16)
        # ones lane for the softmax denominator; col 65 is zero pad
        nc.gpsimd.memset(v_sb[:, :, :, 64:VW], 0.0)
        nc.gpsimd.memset(v_sb[:, :, :, 64:65], 1.0)
        if has_bias:
            keepb = main.tile([128, S], F32)
            keepT = main.tile([128, NB], F32)
            nc.gpsimd.dma_start(
                out=keepb,
                in_=bass.AP(tensor=keep.ap().tensor, offset=0, ap=[[0, 128], [1, S]]),
            )
            nc.gpsimd.dma_start(
                out=keepT,
                in_=bass.AP(tensor=keep.ap().tensor, offset=0, ap=[[1, 128], [128, NB]]),
            )

        # ---- input DMAs: (wqv_i, x-half0_i) interleaved, mask, wo, x-half1 ----
        for i in range(KT):
            nc.sync.dma_start(out=wqv_t[i], in_=wqvT[128 * i : 128 * i + _rh(i), :])
            nc.sync.dma_start(
                out=xt[i][:, 0:1024], in_=xT[128 * i : 128 * i + _rh(i), 0:1024]
            )
        nc.sync.dma_start(out=mk, in_=mask01[:, :])
        for c in range(2):
            nc.sync.dma_start(out=wo_t[c], in_=woT[128 * c : 128 * (c + 1), :])
        for i in range(KT):
            nc.sync.dma_start(
                out=xt[i][:, 1024:2048], in_=xT[128 * i : 128 * i + _rh(i), 1024:2048]
            )

        def touch(t_ap):
            # zero-cost PE toucher: absorbs one DMA semaphore in PE order
            nc.tensor.ldweights(t_ap[:, 0:2])

        # v_ps lives OUTSIDE the phase-1 pool scope so v_proj(0) does not
        # wait on the phase-1 pool-release boundary (which depends on the
        # last qk eviction).
        vps = es.enter_context(tc.tile_pool(name="v_ps", bufs=1, space="PSUM"))

        # ================= phase 1: qk^T projection =================
        # Tile-major over the first SPLIT contraction tiles (matches the PE
        # to the DMA arrival cadence), then c-major over the rest so each
        # chain's eviction hides behind the next chain's matmuls. Chain c3
        # is single-bank (its two token-quarter sub-chains run back to
        # back), keeping the phase-1 pool at 7 banks.
        SPLIT = 4
        with tc.tile_pool(name="qk0_ps", bufs=1, space="PSUM") as qk0ps:
            for half in range(2):
                tiles = [
                    qk0ps.tile([128, 2, 512], F32, name=f"qkp{c}_{half}", tag=f"c{c}")
                    for c in range(3)
                ]
                t23 = {
                    (3, 0): qk0ps.tile(
                        [128, 512], F32, name=f"qkp3_{half}_0", tag="c3"
                    )
                }

                def get_dst(c, sub):
                    if c < 3:
                        return tiles[c][:, sub, :]
                    if (c, sub) not in t23:
                        t23[(c, sub)] = qk0ps.tile(
                            [128, 512], F32, name=f"qkp{c}_{half}_{sub}", tag=f"c{c}"
                        )
                    return t23[(c, sub)][:, :]

                def qk_mm1(c, sub, i):
                    tq = 2 * half + sub
                    nc.tensor.matmul(
                        get_dst(c, sub),
                        wqv_t[i][:, 128 * c : 128 * (c + 1)],
                        xt[i][:, 512 * tq : 512 * (tq + 1)],
                        start=(i == 0),
                        stop=(i == KT - 1),
                    )

                def touch_i(i):
                    if half == 0:
                        touch(wqv_t[i])
                        touch(xt[i])
                    else:
                        touch(xt[i][:, 1024:1026])

                def evict(c, n, sub=None):
                    if c >= 3:
                        dst3 = qk[c][
                            :, 1024 * half + 512 * sub : 1024 * half + 512 * (sub + 1)
                        ]
                        src = t23[(c, sub)][:, :]
                    else:
                        dst = qk[c][:, 1024 * half : 1024 * (half + 1)]
                        dst3 = dst.rearrange("p (a b) -> p a b", a=2)
                        src = tiles[c][:, :, :]
                    if n % 2 == 0:
                        nc.scalar.activation(dst3, src, func=COPY)
                    else:
                        nc.vector.tensor_copy(dst3, src)
                    if has_bias:
                        lo = 1024 * half + (512 * sub if c >= 3 else 0)
                        w = 512 if c >= 3 else 1024
                        nc.vector.tensor_mul(
                            qk[c][:, lo : lo + w], qk[c][:, lo : lo + w],
                            keepb[:, lo : lo + w],
                        )

                for i in range(SPLIT):
                    touch_i(i)
                    for c in range(3):
                        qk_mm1(c, 0, i)
                        qk_mm1(c, 1, i)
                    qk_mm1(3, 0, i)
                if half == 0:
                    touch(mk)
                    touch(wo_t[0])
                    touch(wo_t[1])
                # c-major remainder; c3-sub0 finishes early so its bank's
                # eviction drains behind c2/c1's matmuls and c3-sub1 never
                # stalls; evictions alternate ACT/DVE
                for i in range(SPLIT, KT):
                    touch_i(i)
                    qk_mm1(0, 0, i)
                    qk_mm1(0, 1, i)
                evict(0, 0)
                for i in range(SPLIT, KT):
                    qk_mm1(3, 0, i)
                evict(3, 1, sub=0)
                for i in range(SPLIT, KT):
                    qk_mm1(2, 0, i)
                    qk_mm1(2, 1, i)
                evict(2, 0)
                for i in range(SPLIT, KT):
                    qk_mm1(1, 0, i)
                    qk_mm1(1, 1, i)
                evict(1, 1)
                for i in range(KT):
                    qk_mm1(3, 1, i)
                evict(3, 0, sub=1)

        # mask AP broadcast across the 4 heads of p_sb
        def mk_bcast(moff, qw):
            a = mk[:, moff : moff + qw]
            return bass.AP(
                tensor=a.tensor, offset=a.offset, ap=[a.ap[0], [0, H_LOC], a.ap[1]]
            )

        # outer-thirds mask AP: [128, H_LOC, 2, 128] view of mk cols
        # {moff..moff+128, moff+256..moff+384} broadcast across heads
        def mk_bcast_outer(moff):
            a = mk[:, moff : moff + 384]
            return bass.AP(
                tensor=a.tensor,
                offset=a.offset,
                ap=[a.ap[0], [0, H_LOC], [256, 2], [1, 128]],
            )

        # ================= phase 2: v proj + banded attention =================
        with tc.tile_pool(name="sc_ps", bufs=2, space="PSUM") as scps, tc.tile_pool(
            name="av_ps", bufs=1, space="PSUM"
        ) as avps, tc.tile_pool(name="op_ps", bufs=1, space="PSUM") as opps:

            def v_proj(b2):
                vp = vps.tile([128, V_CH], F32, name=f"vp{b2}", tag="vp")
                for i in range(KT):
                    nc.tensor.matmul(
                        vp[:, :],
                        xt[i][:, 128 * b2 : 128 * (b2 + 1)],
                        wqv_t[i][:, QK_CH : QK_CH + V_CH],
                        start=(i == 0),
                        stop=(i == KT - 1),
                    )
                dst = v_sb[:, b2, :, 0:64]
                src = vp[:, :].rearrange("p (h d) -> p h d", d=64)
                if has_bias:
                    nc.vector.tensor_scalar_mul(dst, src, keepT[:, b2 : b2 + 1])
                else:
                    nc.vector.tensor_copy(dst, src)

            P = {}

            def scores_kb(kb):
                qlo = max(0, 128 * (kb - 1))
                qhi = min(S, 128 * (kb + 2))
                qw = qhi - qlo
                moff = qlo - 128 * (kb - 1)
                p_sb = wk.tile([128, H_LOC, 384], BF16, name=f"p{kb}", tag="p")
                for pair in range(2):
                    sc = scps.tile([128, 2, 512], F32, name=f"sc{kb}_{pair}", tag="sc")
                    for sub in range(2):
                        h = 2 * pair + sub
                        ct = 2 + h // 2
                        pbase = 64 * (h % 2)
                        nc.tensor.matmul(
                            sc[:, sub, 0:qw],
                            qk[ct][pbase : pbase + 64, 128 * kb : 128 * (kb + 1)],
                            qk[h // 2][pbase : pbase + 64, qlo:qhi],
                            start=True,
                            stop=True,
                        )
                    nc.scalar.activation(
                        p_sb[:, 2 * pair : 2 * pair + 2, 0:qw],
                        sc[:, :, 0:qw],
                        func=EXP,
                        scale=0.125,
                    )
                if qw == 384:
                    # only the outer thirds of the q window can be out of band
                    psl = p_sb[:, :, :]
                    pap = bass.AP(
                        tensor=psl.tensor,
                        offset=psl.offset,
                        ap=[psl.ap[0], [384, H_LOC], [256, 2], [1, 128]],
                    )
                    nc.vector.tensor_mul(pap, pap, mk_bcast_outer(0))
                else:
                    nc.vector.tensor_mul(
                        p_sb[:, :, 0:qw], p_sb[:, :, 0:qw], mk_bcast(moff, qw)
                    )
                P[kb] = p_sb

            AVS = {}
            VTS = {}

            def do_av(qblk, tail=False, pool=None):
                kbs = [k2 for k2 in (qblk - 1, qblk, qblk + 1) if 0 <= k2 < NB]
                pool = pool or avps
                a = pool.tile(
                    [128, H_LOC, VW], F32, name=f"av{qblk}",
                    tag="vp" if pool is vps else "av",
                )
                for h in range(H_LOC):
                    for idx, k2 in enumerate(kbs):
                        off = 128 * qblk - max(0, 128 * (k2 - 1))
                        nc.tensor.matmul(
                            a[:, h, :],
                            P[k2][:, h, off : off + 128],
                            v_sb[:, k2, h, :],
                            start=(idx == 0),
                            stop=(idx == len(kbs) - 1),
                        )
                recip = wk2.tile([128, H_LOC, 1], F32, name=f"rc{qblk}", tag="rc")
                nc.vector.reciprocal(recip, a[:, :, 64:65])
                vals = wk2.tile([128, H_LOC, 64], BF16, name=f"vl{qblk}", tag="vl")
                nsplit = 2 if tail else H_LOC
                for h in range(nsplit):
                    nc.vector.tensor_scalar_mul(
                        vals[:, h, :], a[:, h, 0:64], recip[:, h, :]
                    )
                for h in range(nsplit, H_LOC):
                    nc.scalar.activation(
                        vals[:, h, :], a[:, h, 0:64], func=COPY, scale=recip[:, h, :]
                    )
                AVS[qblk] = vals
                if qblk < NB - 3:
                    # mid-loop blocks: transpose via the DMA XBAR (frees the
                    # PE transposes + DVE copy; the pipeline slack hides the
                    # DMA latency)
                    v2d = vals.rearrange("p h d -> p (h d)")
                    vT = wk2.tile([128, 2, 128], BF16, name=f"vTd{qblk}", tag="vT")
                    for c2 in range(2):
                        nc.sync.dma_start(
                            out=vT[:, c2, :],
                            in_=v2d[:, 128 * c2 : 128 * (c2 + 1)],
                            transpose=True,
                        )
                    VTS[qblk] = vT

            def do_out(qblk, pool=None, split_evict=False, tp_pool=None,
                       evict_dve=False):
                vals = AVS.pop(qblk)
                pool = pool or opps
                v2d = vals.rearrange("p h d -> p (h d)")
                vT = VTS.pop(qblk, None)
                dma_tp = vT is not None
                if dma_tp:
                    op = pool.tile(
                        [128, 2, 512], F32, name=f"op{qblk}",
                        tag="sc" if pool is scps else "op",
                    )
                else:
                    vT = wk2.tile([128, 2, 128], BF16, name=f"vT{qblk}", tag="vT")
                if dma_tp:
                    pass
                elif tp_pool is not None:
                    # tail blocks: transposes go to a freed psum bank so they
                    # start right after normalize instead of waiting the
                    # o_proj ring to recycle
                    tpt = tp_pool.tile(
                        [128, 2, 128], BF16, name=f"tpt{qblk}",
                        tag="vp" if tp_pool is vps else "av",
                    )
                    for c2 in range(2):
                        nc.tensor.transpose(
                            tpt[:, c2, :], v2d[:, 128 * c2 : 128 * (c2 + 1)],
                            ident[:, :],
                        )
                    nc.vector.tensor_copy(vT, tpt)
                    op = pool.tile(
                        [128, 2, 512], F32, name=f"op{qblk}",
                        tag="sc" if pool is scps else "op",
                    )
                else:
                    op = pool.tile(
                        [128, 2, 512], F32, name=f"op{qblk}",
                        tag="sc" if pool is scps else "op",
                    )
                    for c2 in range(2):
                        nc.tensor.transpose(
                            op[:, 0, 64 * c2 : 64 * (c2 + 1)].bitcast(BF16),
                            v2d[:, 128 * c2 : 128 * (c2 + 1)],
                            ident[:, :],
                        )
                    nc.vector.tensor_copy(
                        vT.rearrange("p a b -> p (a b)"), op[:, 0, 0:128].bitcast(BF16)
                    )
                for n2 in (1, 0):  # bank 1 first: bank 0 waits the vT read
                    for c2 in range(2):
                        nc.tensor.matmul(
                            op[:, n2, :],
                            vT[:, c2, :],
                            wo_t[c2][:, 512 * n2 : 512 * (n2 + 1)],
                            start=(c2 == 0),
                            stop=(c2 == 1),
                        )
                ot = wk2.tile([128, 2, 512], BF16, name=f"ot{qblk}", tag="ot")
                if split_evict:
                    # last block: evict + store per bank so the final DMA
                    # starts as early as possible
                    for n2 in (1, 0):
                        nc.scalar.activation(ot[:, n2, :], op[:, n2, :], func=COPY)
                        nc.sync.dma_start(
                            out=out[
                                128 * qblk : 128 * (qblk + 1),
                                512 * n2 : 512 * (n2 + 1),
                            ],
                            in_=ot[:, n2, :],
                        )
                else:
                    if evict_dve:
                        nc.vector.tensor_copy(ot, op[:, :, :])
                    else:
                        nc.scalar.activation(ot, op[:, :, :], func=COPY)
                    nc.sync.dma_start(
                        out=out[128 * qblk : 128 * (qblk + 1), :],
                        in_=ot.rearrange("p a b -> p (a b)"),
                    )

            # ---- software pipeline ----
            for t in range(NB):
                if t >= 1:
                    scores_kb(t - 1)
                if t >= 3:
                    do_out(t - 3)
                if t >= 2:
                    do_av(t - 2)
                v_proj(t)
            # tail: the scores psum ring is free once scores(15) drains, so
            # alternate the last o_proj blocks onto it — tail chains run
            # two-wide instead of serializing on the single op buffer.
            scores_kb(NB - 1)
            do_av(NB - 2, tail=True)
            do_out(NB - 3, pool=scps)
            do_av(NB - 1, tail=True)
            do_out(NB - 2)
            do_out(NB - 1, pool=scps, split_evict=True)

    return nc


def _get_nc(has_bias=False):
    key = ("nc", has_bias)
    if key not in _CACHE:
        nc = _build_nc(has_bias)
        nc.finalize()
        _CACHE[key] = nc
    return _CACHE[key]


def _prep_in_maps(x, padding_mask, Wqkv, bqkv, Wo, bo, has_bias=None):
    f32 = np.float32
    x = np.asarray(x, dtype=f32)
    pm = np.asarray(padding_mask)
    Wqkv = np.asarray(Wqkv, dtype=f32)
    bqkv = np.asarray(bqkv, dtype=f32)
    Wo = np.asarray(Wo, dtype=f32)
    if has_bias is None:
        has_bias = bool(np.any(bqkv))

    import ml_dtypes

    bf16 = ml_dtypes.bfloat16

    IN_ROWS = IN_DIM + (2 if has_bias else 0)

    # band mask tile: mask[k, qr] = 1 iff 0 <= qr - k <= 256
    k_idx = np.arange(128)[:, None]
    q_idx = np.arange(384)[None, :]
    d = q_idx - k_idx
    mask01 = ((d >= 0) & (d <= WINDOW)).astype(bf16)

    xT_b = []
    keep_b = []
    for b in range(B):
        xz = x[b].copy()
        xz[pm[b] != 0] = 0.0  # zero padded tokens on the host
        aug = np.zeros((IN_ROWS, S), dtype=bf16)
        aug[:IN_DIM] = xz.T.astype(bf16)
        if has_bias:
            aug[IN_DIM] = bf16(1.0)
        xT_b.append(aug)
        keep_b.append((pm[b] == 0).astype(f32).reshape(1, S))

    in_maps = []
    for c in range(N_CORES):
        b = c // 4
        g = c % 4
        heads = [4 * g + j for j in range(H_LOC)]
        q_rows = np.concatenate([Wqkv[192 * h : 192 * h + 64] for h in heads])
        k_rows = np.concatenate([Wqkv[192 * h + 64 : 192 * h + 128] for h in heads])
        v_rows = np.concatenate([Wqkv[192 * h + 128 : 192 * h + 192] for h in heads])

        wqvT = np.zeros((IN_ROWS, QK_CH + V_CH), dtype=bf16)
        wqvT[:IN_DIM] = np.concatenate([q_rows, k_rows, v_rows]).T.astype(bf16)
        if has_bias:
            bq = np.concatenate([bqkv[192 * h : 192 * h + 64] for h in heads])
            bk = np.concatenate([bqkv[192 * h + 64 : 192 * h + 128] for h in heads])
            bv = np.concatenate([bqkv[192 * h + 128 : 192 * h + 192] for h in heads])
            wqvT[IN_DIM] = np.concatenate([bq, bk, bv]).astype(bf16)
        woT = np.ascontiguousarray(Wo[:, 256 * g : 256 * (g + 1)].T).astype(bf16)

        im = {
            "xT": xT_b[b],
            "wqvT": wqvT,
            "woT": woT,
            "mask01": mask01,
        }
        if has_bias:
            im["keep"] = keep_b[b]
        in_maps.append(im)
    return in_maps


def kernel(x, padding_mask, Wqkv, bqkv, Wo, bo):
    from concourse.bass_utils import run_bass_kernel_spmd

    has_bias = bool(np.any(np.asarray(bqkv)))
    nc = _get_nc(has_bias)
    in_maps = _prep_in_maps(x, padding_mask, Wqkv, bqkv, Wo, bo, has_bias)
    trace = bool(int(os.environ.get("KERNEL_TRACE", "0")))
    res = run_bass_kernel_spmd(nc, in_maps, list(range(N_CORES)), trace=trace)
    LAST["exec_time_ns"] = res.exec_time_ns
    LAST["results"] = res

    bo = np.asarray(bo, dtype=np.float32)
    out = np.zeros((B, S, EMBED), dtype=np.float32)
    for c in range(N_CORES):
        out[c // 4] += np.asarray(res.results[c]["out"], dtype=np.float32)
    out += bo[None, None, :]
    return out
